# revision 29
# baseline (speedup 1.0000x reference)
"""Trainium2 Bass kernel for CachedMultiheadAttention (sliding-window + ALiBi).

Sharding: 8 cores = 2 batches x 4 head-quartets. Core c handles batch c//4 and
heads [4*(c%4), 4*(c%4)+4). Each core computes QKV projection for its heads,
banded attention (causal + 512 window + ALiBi), and a partial out-projection
over its heads' 256 embedding columns. Host sums the 4 partials per batch.

v3 (all-bf16, PE-dense, batched normalization):
  - inputs pre-cast to bf16 on host (half the HBM traffic); x^T/w loads split
    into many small DMAs so the first matmul inputs land on many rings fast.
  - V projected directly into natural [t, d] layout (lhsT = x^T block), no PE
    transposes; ones column in vnat gives softmax rowsums via the AV matmul.
  - S^T strips: PE (bf16) -> exp on ACT -> multiply by precomputed band*ALiBi
    bias tile (split across DVE / gpsimd).
  - AV results + rowsum rows staged to SBUF immediately (frees PSUM, no
    deadlock), then ONE plain DVE reciprocal per head-pair on the batched
    [8,512] rowsum tile -- no scalar-table thrash, no custom-DVE ops (which
    silently no-op on this rig).
  - out-projection (bf16) per column group right after its normalize, so PE
    stays dense to the end; partial over 256 local e-rows, host-summed.
"""
import math

import numpy as np
import ml_dtypes

import concourse.bass as bass
import concourse.tile as tile
from concourse import bacc, mybir
from concourse.bass_utils import run_bass_kernel_spmd

F32 = mybir.dt.float32
F32R = mybir.dt.float32r
BF16 = mybir.dt.bfloat16

B, T, E, H, HD, W = 2, 2048, 1024, 16, 64, 512
NCORES = 8
HL = 4                # local heads per core
NT = T // 128         # 16 t-blocks

_CACHE = {}


def _get_slopes(n):
    def p2(m):
        start = 2 ** (-(2 ** (-(math.log2(m) - 3))))
        return [start * start**i for i in range(m)]
    if math.log2(n) % 1 == 0:
        return p2(n)
    c = 2 ** math.floor(math.log2(n))
    return p2(c) + _get_slopes(2 * c)[0::2][: n - c]


def _build():
    nc = bacc.Bacc("TRN2", target_bir_lowering=False, debug=False, num_devices=NCORES)
    xT = nc.dram_tensor("xT", [8, 128, T], BF16, kind="ExternalInput").ap()
    wqkv = nc.dram_tensor("wqkv", [8, 128, 768], BF16, kind="ExternalInput").ap()
    wo = nc.dram_tensor("wo", [2, 128, E], BF16, kind="ExternalInput").ap()
    biasd = nc.dram_tensor("biasd", [HL, 128, 640], BF16, kind="ExternalInput").ap()
    outT = nc.dram_tensor("outT", [8, 128, T], BF16, kind="ExternalOutput").ap()

    with tile.TileContext(nc) as tc:
        with (
            tc.tile_pool(name="singles", bufs=1) as singles,
            tc.tile_pool(name="ptp", bufs=3) as ptp,
            tc.tile_pool(name="sprep", bufs=3) as sprep,
            tc.tile_pool(name="aostp", bufs=1) as aostp,
            tc.tile_pool(name="smallp", bufs=2) as smallp,
            tc.tile_pool(name="evp", bufs=2) as evp,
            tc.tile_pool(name="mm", bufs=3, space="PSUM") as mmp,
            tc.tile_pool(name="aop", bufs=3, space="PSUM") as aop,
            tc.tile_pool(name="bcps", bufs=2, space="PSUM") as bcps,
        ):
            dmae = [nc.sync, nc.scalar]

            # --- one-time loads: first-needed first, split fine so the lead
            # chunks land on many DMA rings in parallel ---
            wqkv_sb = singles.tile([128, 8, 768], BF16)
            xT_sb = singles.tile([128, 8, T], BF16)
            di = 0
            for ec in range(8):
                for q4 in range(4):           # wqkv[ec] in 4 pieces (48 KB)
                    dmae[di % 2].dma_start(
                        wqkv_sb[:, ec, q4 * 192:(q4 + 1) * 192],
                        wqkv[ec, :, q4 * 192:(q4 + 1) * 192])
                    di += 1
                for h2 in range(2):           # x tb0 chunk in 2 pieces (64 KB)
                    dmae[di % 2].dma_start(
                        xT_sb[:, ec, h2 * 256:(h2 + 1) * 256],
                        xT[ec, :, h2 * 256:(h2 + 1) * 256])
                    di += 1
            for tb in range(1, 4):
                for ec in range(8):
                    dmae[di % 2].dma_start(
                        xT_sb[:, ec, tb * 512:(tb + 1) * 512],
                        xT[ec, :, tb * 512:(tb + 1) * 512])
                    di += 1
            bias_sb = singles.tile([128, HL, 640], BF16)
            for hl in range(HL):
                dmae[hl % 2].dma_start(bias_sb[:, hl, :], biasd[hl])
            wo_sb = singles.tile([128, 2, E], BF16)
            nc.sync.dma_start(wo_sb[:], wo.rearrange("c p f -> p c f"))

            qkvT = singles.tile([128, 4, T], BF16)   # slots: Qp0 Qp1 Kp0 Kp1
            vnat = singles.tile([128, HL, NT, HD + 1], BF16)
            nc.gpsimd.memset(vnat[:], 1.0)           # ones column at [...,64]
            ao2T = singles.tile([128, 2, T], BF16)   # normalized AO^T
            ones128 = singles.tile([128, 512], F32)
            nc.gpsimd.memset(ones128[:], 1.0)
            rsab = [singles.tile([97, 512], F32, name=f"rsab{x}") for x in range(4)]
            for x in range(4):
                nc.gpsimd.memset(rsab[x][:], 1.0)
            rrab = [singles.tile([97, 512], F32, name=f"rrab{x}") for x in range(4)]
            onesr = singles.tile([1, 64], F32)
            nc.gpsimd.memset(onesr[:], 1.0)

            # --- phase 1: Q^T/K^T projection + V natural-layout projection ---
            for tb in range(4):
                for m in range(4):
                    pt = mmp.tile([128, 512], F32, tag="mm512")
                    for ec in range(8):
                        nc.tensor.matmul(
                            pt[:],
                            lhsT=wqkv_sb[:, ec, m * 128:(m + 1) * 128],
                            rhs=xT_sb[:, ec, tb * 512:(tb + 1) * 512],
                            start=(ec == 0), stop=(ec == 7),
                        )
                    if m % 2 == 0:
                        nc.scalar.copy(qkvT[:, m, tb * 512:(tb + 1) * 512], pt[:])
                    else:
                        nc.vector.tensor_copy(
                            qkvT[:, m, tb * 512:(tb + 1) * 512], pt[:])
                for tl in range(4):
                    tk = tb * 4 + tl
                    vt = mmp.tile([128, 4, HD], F32, tag="mm512")
                    for ec in range(8):
                        nc.tensor.matmul(
                            vt[:],
                            lhsT=xT_sb[:, ec, tk * 128:(tk + 1) * 128],
                            rhs=wqkv_sb[:, ec, 512:768],
                            start=(ec == 0), stop=(ec == 7),
                        )
                    nc.vector.tensor_copy(vnat[:, :, tk, 0:HD], vt[:])

            # --- phase 2: attention, head-pair interleaved ---
            # Per sq: S strips -> AV (+staging). sq0's reciprocal/normalize
            # chain is emitted AFTER sq1's strips so the vector queue never
            # blocks the strip pipeline; the broadcast of 1/rowsum is a rank-1
            # PE matmul into PSUM (gpsimd stays single-op-type); phase 3 last.
            pend = {}

            def strips(sq, pths):
                for jb in range(NT):
                    nq = min(5, NT - jb)
                    qw = nq * 128
                    w0 = min(qw, 512)
                    for hh in range(2):
                        h = 2 * sq + hh
                        r0 = hh * 64
                        pth = pths[hh]
                        praw = sprep.tile([128, 640], BF16, tag="praw")
                        s5 = mmp.tile([128, 512], F32, tag="mm512")
                        nc.tensor.matmul(
                            s5[:, 0:w0],
                            lhsT=qkvT[r0:r0 + 64, 2 + sq, jb * 128:(jb + 1) * 128],
                            rhs=qkvT[r0:r0 + 64, sq, jb * 128:jb * 128 + w0],
                            start=True, stop=True,
                        )
                        nc.scalar.activation(
                            out=praw[:, 0:w0], in_=s5[:, 0:w0],
                            func=mybir.ActivationFunctionType.Exp,
                        )
                        if qw > 512:
                            s1 = mmp.tile([128, 128], F32, tag="mm512")
                            nc.tensor.matmul(
                                s1[:],
                                lhsT=qkvT[r0:r0 + 64, 2 + sq, jb * 128:(jb + 1) * 128],
                                rhs=qkvT[r0:r0 + 64, sq, jb * 128 + 512:jb * 128 + qw],
                                start=True, stop=True,
                            )
                            nc.scalar.activation(
                                out=praw[:, 512:qw], in_=s1[:],
                                func=mybir.ActivationFunctionType.Exp,
                            )
                        # P = exp(S) * exp(bias): band mask + ALiBi
                        eng = nc.vector if hh == 0 else nc.gpsimd
                        eng.tensor_tensor(
                            out=pth[:, jb, 0:qw], in0=praw[:, 0:qw],
                            in1=bias_sb[:, h, 0:qw], op=mybir.AluOpType.mult,
                        )

            def av_stage(sq, pths):
                aostg = aostp.tile([64, 8, 512], BF16, tag="aostg")
                for g in range(4):
                    for hh in range(2):
                        h = 2 * sq + hh
                        pth = pths[hh]
                        ao = aop.tile([65, 512], F32, tag="ao")
                        jbs = [4 * g] + [jb for jb in range(max(0, 4 * g - 4), 4 * g + 4)
                                         if jb != 4 * g]
                        for i, jb in enumerate(jbs):
                            qb_lo = max(4 * g, jb)
                            qb_hi = min(4 * g + 3, jb + 4)
                            wdt = (qb_hi - qb_lo + 1) * 128
                            ao_off = (qb_lo - 4 * g) * 128
                            p_off = (qb_lo - jb) * 128
                            nc.tensor.matmul(
                                ao[:, ao_off:ao_off + wdt],
                                lhsT=vnat[:, h, jb, :],
                                rhs=pth[:, jb, p_off:p_off + wdt],
                                start=(i == 0), stop=(i == len(jbs) - 1),
                                skip_group_check=True,
                            )
                        nc.vector.tensor_tensor(
                            out=rsab[2 * sq + hh][32 * g:32 * g + 1, :],
                            in0=ao[64:65, :], in1=ones128[0:1, :],
                            op=mybir.AluOpType.mult)
                        if hh == 0:
                            nc.scalar.copy(aostg[:, 2 * g, :], ao[0:64, :])
                        else:
                            nc.vector.tensor_copy(aostg[:, 2 * g + 1, :], ao[0:64, :])
                pend[sq] = aostg

            def chain_norm(sq):
                aostg = pend.pop(sq)
                # batched reciprocals, then per slot: row extract -> rank-1
                # PE broadcast into PSUM -> normalize multiply
                nc.vector.reciprocal(rrab[2 * sq][:], rsab[2 * sq][:])
                nc.vector.reciprocal(rrab[2 * sq + 1][:], rsab[2 * sq + 1][:])
                rrgs = []
                for g in range(4):
                    for hh in range(2):
                        rrg = smallp.tile([1, 512], F32R, tag="rrg", bufs=8)
                        nc.vector.tensor_tensor(
                            out=rrg[:], in0=rrab[2 * sq + hh][32 * g:32 * g + 1, :],
                            in1=ones128[32 * g:32 * g + 1, :],
                            op=mybir.AluOpType.mult)
                        rrgs.append(rrg)
                for g in range(4):
                    for hh in range(2):
                        r0 = hh * 64
                        bc = bcps.tile([64, 512], F32, tag="bc")
                        nc.tensor.matmul(
                            bc[:], lhsT=onesr[:].bitcast(F32R),
                            rhs=rrgs[2 * g + hh][:],
                            start=True, stop=True,
                        )
                        nc.vector.tensor_tensor(
                            out=ao2T[r0:r0 + 64, sq, g * 512:(g + 1) * 512],
                            in0=aostg[:, 2 * g + hh, :], in1=bc[:],
                            op=mybir.AluOpType.mult,
                        )

            pths0 = [ptp.tile([128, NT, 640], BF16, tag="pth", name="ptha0"),
                     ptp.tile([128, NT, 640], BF16, tag="pth", name="pthb0")]
            strips(0, pths0)
            av_stage(0, pths0)
            pths1 = [ptp.tile([128, NT, 640], BF16, tag="pth", name="ptha1"),
                     ptp.tile([128, NT, 640], BF16, tag="pth", name="pthb1")]
            strips(1, pths1)
            chain_norm(0)
            av_stage(1, pths1)
            chain_norm(1)

            # --- phase 3: out projection (bf16, partial over local e-rows) ---
            for tb in range(4):
                for fc in range(8):
                    po = mmp.tile([128, 512], F32, tag="mm512")
                    for c2 in range(2):
                        nc.tensor.matmul(
                            po[:],
                            lhsT=wo_sb[:, c2, fc * 128:(fc + 1) * 128],
                            rhs=ao2T[:, c2, tb * 512:(tb + 1) * 512],
                            start=(c2 == 0), stop=(c2 == 1),
                        )
                    ev = evp.tile([128, 512], BF16, tag="ev")
                    if fc % 2 == 0:
                        nc.vector.tensor_copy(ev[:], po[:])
                    else:
                        nc.scalar.copy(ev[:], po[:])
                    eng = nc.sync if fc % 2 == 0 else nc.scalar
                    eng.dma_start(outT[fc, :, tb * 512:(tb + 1) * 512], ev[:])

    nc.compile()
    return nc


def _host_inputs(query, in_proj_weight, out_proj_weight):
    """Per-core input maps (numpy only)."""
    slopes = np.asarray(_get_slopes(H), np.float32)
    q32 = np.asarray(query, np.float32)
    w_in = np.asarray(in_proj_weight, np.float32)
    w_out = np.asarray(out_proj_weight, np.float32)

    # band+alibi bias tiles, shift-invariant per head: [h, jj, cc]
    jj = np.arange(128)[:, None]
    cc = np.arange(640)[None, :]
    allowed = (cc >= jj) & (cc - jj <= W)
    in_maps = []
    for c in range(NCORES):
        b, hq = divmod(c, 4)
        heads = np.arange(4 * hq, 4 * hq + HL)
        rows = (heads[:, None] * HD + np.arange(HD)[None, :]).reshape(-1)  # 256 rows
        wq = w_in[rows, :] * (1.0 / math.sqrt(HD))
        wk = w_in[E + rows, :]
        wv = w_in[2 * E + rows, :]
        w_loc = np.concatenate([wq, wk, wv], axis=0)          # [768, E]
        wqkv = np.ascontiguousarray(
            w_loc.T.reshape(8, 128, 768)).astype(ml_dtypes.bfloat16)

        xT = np.ascontiguousarray(
            q32[b].T.reshape(8, 128, T)).astype(ml_dtypes.bfloat16)

        wo_loc = np.ascontiguousarray(
            w_out[:, rows].T.reshape(2, 128, E)).astype(ml_dtypes.bfloat16)

        biasd = np.empty((HL, 128, 640), ml_dtypes.bfloat16)
        for hl in range(HL):
            s = slopes[4 * hq + hl]
            eb = np.where(allowed, np.exp(-s * (cc - jj).astype(np.float64)), 0.0)
            biasd[hl] = eb.astype(ml_dtypes.bfloat16)

        in_maps.append({"xT": xT, "wqkv": wqkv, "wo": wo_loc, "biasd": biasd})
    return in_maps


def _assemble(results):
    out = np.zeros((B, T, E), np.float32)
    for c in range(NCORES):
        b = c // 4
        part = np.asarray(results[c]["outT"]).astype(np.float32)  # [8,128,T]
        out[b] += part.reshape(E, T).T
    return out


def kernel(query, in_proj_weight, out_proj_weight, num_heads, sliding_window_size):
    assert int(num_heads) == H and int(sliding_window_size) == W
    assert query.shape == (B, T, E)
    if "nc" not in _CACHE:
        _CACHE["nc"] = _build()
    in_maps = _host_inputs(query, in_proj_weight, out_proj_weight)
    res = run_bass_kernel_spmd(_CACHE["nc"], in_maps, list(range(NCORES))).results
    return _assemble(res)


# revision 30
# speedup vs baseline: 1.0284x; 1.0284x over previous
"""Trainium2 Bass kernel for CachedMultiheadAttention (sliding-window + ALiBi).

Sharding: 8 cores = 2 batches x 4 head-quartets. Core c handles batch c//4 and
heads [4*(c%4), 4*(c%4)+4). Each core computes QKV projection for its heads,
banded attention (causal + 512 window + ALiBi), and a partial out-projection
over its heads' 256 embedding columns. Host sums the 4 partials per batch.

v3 (all-bf16, PE-dense, batched normalization):
  - inputs pre-cast to bf16 on host (half the HBM traffic); x^T/w loads split
    into many small DMAs so the first matmul inputs land on many rings fast.
  - V projected directly into natural [t, d] layout (lhsT = x^T block), no PE
    transposes; ones column in vnat gives softmax rowsums via the AV matmul.
  - S^T strips: PE (bf16) -> exp on ACT -> multiply by precomputed band*ALiBi
    bias tile (split across DVE / gpsimd).
  - AV results + rowsum rows staged to SBUF immediately (frees PSUM, no
    deadlock), then ONE plain DVE reciprocal per head-pair on the batched
    [8,512] rowsum tile -- no scalar-table thrash, no custom-DVE ops (which
    silently no-op on this rig).
  - out-projection (bf16) per column group right after its normalize, so PE
    stays dense to the end; partial over 256 local e-rows, host-summed.
"""
import math

import numpy as np
import ml_dtypes

import concourse.bass as bass
import concourse.tile as tile
from concourse import bacc, mybir
from concourse.bass_utils import run_bass_kernel_spmd

F32 = mybir.dt.float32
F32R = mybir.dt.float32r
BF16 = mybir.dt.bfloat16

B, T, E, H, HD, W = 2, 2048, 1024, 16, 64, 512
NCORES = 8
HL = 4                # local heads per core
NT = T // 128         # 16 t-blocks

_CACHE = {}


def _get_slopes(n):
    def p2(m):
        start = 2 ** (-(2 ** (-(math.log2(m) - 3))))
        return [start * start**i for i in range(m)]
    if math.log2(n) % 1 == 0:
        return p2(n)
    c = 2 ** math.floor(math.log2(n))
    return p2(c) + _get_slopes(2 * c)[0::2][: n - c]


def _build():
    nc = bacc.Bacc("TRN2", target_bir_lowering=False, debug=False, num_devices=NCORES)
    xT = nc.dram_tensor("xT", [8, 128, T], BF16, kind="ExternalInput").ap()
    wqkv = nc.dram_tensor("wqkv", [8, 128, 768], BF16, kind="ExternalInput").ap()
    wo = nc.dram_tensor("wo", [2, 128, E], BF16, kind="ExternalInput").ap()
    biasd = nc.dram_tensor("biasd", [HL, 128, 640], BF16, kind="ExternalInput").ap()
    outT = nc.dram_tensor("outT", [8, 128, T], BF16, kind="ExternalOutput").ap()

    with tile.TileContext(nc) as tc:
        with (
            tc.tile_pool(name="singles", bufs=1) as singles,
            tc.tile_pool(name="ptp", bufs=3) as ptp,
            tc.tile_pool(name="sprep", bufs=3) as sprep,
            tc.tile_pool(name="aostp", bufs=1) as aostp,
            tc.tile_pool(name="smallp", bufs=2) as smallp,
            tc.tile_pool(name="evp", bufs=2) as evp,
            tc.tile_pool(name="mm", bufs=3, space="PSUM") as mmp,
            tc.tile_pool(name="aop", bufs=3, space="PSUM") as aop,
            tc.tile_pool(name="bcps", bufs=2, space="PSUM") as bcps,
        ):
            dmae = [nc.sync, nc.scalar, nc.gpsimd]

            # --- one-time loads: first-needed first, split fine so the lead
            # chunks land on many DMA rings in parallel ---
            wqkv_sb = singles.tile([128, 8, 768], BF16)
            xT_sb = singles.tile([128, 8, T], BF16)
            di = 0
            for ec in range(8):
                for q4 in range(4):           # wqkv[ec] in 4 pieces (48 KB)
                    dmae[di % 3].dma_start(
                        wqkv_sb[:, ec, q4 * 192:(q4 + 1) * 192],
                        wqkv[ec, :, q4 * 192:(q4 + 1) * 192])
                    di += 1
                for h2 in range(2):           # x tb0 chunk in 2 pieces (64 KB)
                    dmae[di % 3].dma_start(
                        xT_sb[:, ec, h2 * 256:(h2 + 1) * 256],
                        xT[ec, :, h2 * 256:(h2 + 1) * 256])
                    di += 1
            for tb in range(1, 4):
                for ec in range(8):
                    dmae[di % 3].dma_start(
                        xT_sb[:, ec, tb * 512:(tb + 1) * 512],
                        xT[ec, :, tb * 512:(tb + 1) * 512])
                    di += 1
            bias_sb = singles.tile([128, HL, 640], BF16)
            for hl in range(HL):
                dmae[hl % 3].dma_start(bias_sb[:, hl, :], biasd[hl])
            wo_sb = singles.tile([128, 2, E], BF16)
            nc.sync.dma_start(wo_sb[:], wo.rearrange("c p f -> p c f"))

            qkvT = singles.tile([128, 4, T], BF16)   # slots: Qp0 Qp1 Kp0 Kp1
            vnat = singles.tile([128, HL, NT, HD + 1], BF16)
            nc.gpsimd.memset(vnat[:], 1.0)           # ones column at [...,64]
            ao2T = singles.tile([128, 2, T], BF16)   # normalized AO^T
            ones128 = singles.tile([128, 512], F32)
            nc.gpsimd.memset(ones128[:], 1.0)
            rsab = [singles.tile([97, 512], F32, name=f"rsab{x}") for x in range(4)]
            for x in range(4):
                nc.gpsimd.memset(rsab[x][:], 1.0)
            rrab = [singles.tile([97, 512], F32, name=f"rrab{x}") for x in range(4)]
            onesr = singles.tile([1, 64], F32)
            nc.gpsimd.memset(onesr[:], 1.0)

            # --- phase 1: Q^T/K^T projection + V natural-layout projection ---
            for tb in range(4):
                for m in range(4):
                    pt = mmp.tile([128, 512], F32, tag="mm512")
                    for ec in range(8):
                        nc.tensor.matmul(
                            pt[:],
                            lhsT=wqkv_sb[:, ec, m * 128:(m + 1) * 128],
                            rhs=xT_sb[:, ec, tb * 512:(tb + 1) * 512],
                            start=(ec == 0), stop=(ec == 7),
                        )
                    if m % 2 == 0:
                        nc.scalar.copy(qkvT[:, m, tb * 512:(tb + 1) * 512], pt[:])
                    else:
                        nc.vector.tensor_copy(
                            qkvT[:, m, tb * 512:(tb + 1) * 512], pt[:])
                for tl in range(4):
                    tk = tb * 4 + tl
                    vt = mmp.tile([128, 4, HD], F32, tag="mm512")
                    for ec in range(8):
                        nc.tensor.matmul(
                            vt[:],
                            lhsT=xT_sb[:, ec, tk * 128:(tk + 1) * 128],
                            rhs=wqkv_sb[:, ec, 512:768],
                            start=(ec == 0), stop=(ec == 7),
                        )
                    nc.vector.tensor_copy(vnat[:, :, tk, 0:HD], vt[:])

            # --- phase 2: attention, head-pair interleaved ---
            # Per sq: S strips -> AV (+staging). sq0's reciprocal/normalize
            # chain is emitted AFTER sq1's strips so the vector queue never
            # blocks the strip pipeline; the broadcast of 1/rowsum is a rank-1
            # PE matmul into PSUM (gpsimd stays single-op-type); phase 3 last.
            pend = {}

            def strips(sq, pths):
                for jb in range(NT):
                    nq = min(5, NT - jb)
                    qw = nq * 128
                    w0 = min(qw, 512)
                    for hh in range(2):
                        h = 2 * sq + hh
                        r0 = hh * 64
                        pth = pths[hh]
                        praw = sprep.tile([128, 640], BF16, tag="praw")
                        s5 = mmp.tile([128, 512], F32, tag="mm512")
                        nc.tensor.matmul(
                            s5[:, 0:w0],
                            lhsT=qkvT[r0:r0 + 64, 2 + sq, jb * 128:(jb + 1) * 128],
                            rhs=qkvT[r0:r0 + 64, sq, jb * 128:jb * 128 + w0],
                            start=True, stop=True,
                        )
                        nc.scalar.activation(
                            out=praw[:, 0:w0], in_=s5[:, 0:w0],
                            func=mybir.ActivationFunctionType.Exp,
                        )
                        if qw > 512:
                            s1 = mmp.tile([128, 128], F32, tag="mm512")
                            nc.tensor.matmul(
                                s1[:],
                                lhsT=qkvT[r0:r0 + 64, 2 + sq, jb * 128:(jb + 1) * 128],
                                rhs=qkvT[r0:r0 + 64, sq, jb * 128 + 512:jb * 128 + qw],
                                start=True, stop=True,
                            )
                            nc.scalar.activation(
                                out=praw[:, 512:qw], in_=s1[:],
                                func=mybir.ActivationFunctionType.Exp,
                            )
                        # P = exp(S) * exp(bias): band mask + ALiBi
                        eng = nc.vector if hh == 0 else nc.gpsimd
                        eng.tensor_tensor(
                            out=pth[:, jb, 0:qw], in0=praw[:, 0:qw],
                            in1=bias_sb[:, h, 0:qw], op=mybir.AluOpType.mult,
                        )

            def av_stage(sq, pths):
                aostg = aostp.tile([64, 8, 512], BF16, tag="aostg")
                for g in range(4):
                    for hh in range(2):
                        h = 2 * sq + hh
                        pth = pths[hh]
                        ao = aop.tile([65, 512], F32, tag="ao")
                        jbs = [4 * g] + [jb for jb in range(max(0, 4 * g - 4), 4 * g + 4)
                                         if jb != 4 * g]
                        for i, jb in enumerate(jbs):
                            qb_lo = max(4 * g, jb)
                            qb_hi = min(4 * g + 3, jb + 4)
                            wdt = (qb_hi - qb_lo + 1) * 128
                            ao_off = (qb_lo - 4 * g) * 128
                            p_off = (qb_lo - jb) * 128
                            nc.tensor.matmul(
                                ao[:, ao_off:ao_off + wdt],
                                lhsT=vnat[:, h, jb, :],
                                rhs=pth[:, jb, p_off:p_off + wdt],
                                start=(i == 0), stop=(i == len(jbs) - 1),
                                skip_group_check=True,
                            )
                        nc.vector.tensor_tensor(
                            out=rsab[2 * sq + hh][32 * g:32 * g + 1, :],
                            in0=ao[64:65, :], in1=ones128[0:1, :],
                            op=mybir.AluOpType.mult)
                        if hh == 0:
                            nc.scalar.copy(aostg[:, 2 * g, :], ao[0:64, :])
                        else:
                            nc.vector.tensor_copy(aostg[:, 2 * g + 1, :], ao[0:64, :])
                pend[sq] = aostg

            def chain_norm(sq):
                aostg = pend.pop(sq)
                # batched reciprocals, then per slot: row extract -> rank-1
                # PE broadcast into PSUM -> normalize multiply
                nc.vector.reciprocal(rrab[2 * sq][:], rsab[2 * sq][:])
                nc.vector.reciprocal(rrab[2 * sq + 1][:], rsab[2 * sq + 1][:])
                rrgs = []
                for g in range(4):
                    for hh in range(2):
                        rrg = smallp.tile([1, 512], F32R, tag="rrg", bufs=8)
                        nc.vector.tensor_tensor(
                            out=rrg[:], in0=rrab[2 * sq + hh][32 * g:32 * g + 1, :],
                            in1=ones128[32 * g:32 * g + 1, :],
                            op=mybir.AluOpType.mult)
                        rrgs.append(rrg)
                for g in range(4):
                    for hh in range(2):
                        r0 = hh * 64
                        bc = bcps.tile([64, 512], F32, tag="bc")
                        nc.tensor.matmul(
                            bc[:], lhsT=onesr[:].bitcast(F32R),
                            rhs=rrgs[2 * g + hh][:],
                            start=True, stop=True,
                        )
                        nc.vector.tensor_tensor(
                            out=ao2T[r0:r0 + 64, sq, g * 512:(g + 1) * 512],
                            in0=aostg[:, 2 * g + hh, :], in1=bc[:],
                            op=mybir.AluOpType.mult,
                        )

            pths0 = [ptp.tile([128, NT, 640], BF16, tag="pth", name="ptha0"),
                     ptp.tile([128, NT, 640], BF16, tag="pth", name="pthb0")]
            strips(0, pths0)
            av_stage(0, pths0)
            pths1 = [ptp.tile([128, NT, 640], BF16, tag="pth", name="ptha1"),
                     ptp.tile([128, NT, 640], BF16, tag="pth", name="pthb1")]
            strips(1, pths1)
            chain_norm(0)
            av_stage(1, pths1)
            chain_norm(1)

            # --- phase 3: out projection (bf16, partial over local e-rows) ---
            for tb in range(4):
                for fc in range(8):
                    po = mmp.tile([128, 512], F32, tag="mm512")
                    for c2 in range(2):
                        nc.tensor.matmul(
                            po[:],
                            lhsT=wo_sb[:, c2, fc * 128:(fc + 1) * 128],
                            rhs=ao2T[:, c2, tb * 512:(tb + 1) * 512],
                            start=(c2 == 0), stop=(c2 == 1),
                        )
                    ev = evp.tile([128, 512], BF16, tag="ev")
                    if fc % 2 == 0:
                        nc.vector.tensor_copy(ev[:], po[:])
                    else:
                        nc.scalar.copy(ev[:], po[:])
                    eng = nc.sync if fc % 2 == 0 else nc.scalar
                    eng.dma_start(outT[fc, :, tb * 512:(tb + 1) * 512], ev[:])

    nc.compile()
    return nc


def _host_inputs(query, in_proj_weight, out_proj_weight):
    """Per-core input maps (numpy only)."""
    slopes = np.asarray(_get_slopes(H), np.float32)
    q32 = np.asarray(query, np.float32)
    w_in = np.asarray(in_proj_weight, np.float32)
    w_out = np.asarray(out_proj_weight, np.float32)

    # band+alibi bias tiles, shift-invariant per head: [h, jj, cc]
    jj = np.arange(128)[:, None]
    cc = np.arange(640)[None, :]
    allowed = (cc >= jj) & (cc - jj <= W)
    in_maps = []
    for c in range(NCORES):
        b, hq = divmod(c, 4)
        heads = np.arange(4 * hq, 4 * hq + HL)
        rows = (heads[:, None] * HD + np.arange(HD)[None, :]).reshape(-1)  # 256 rows
        wq = w_in[rows, :] * (1.0 / math.sqrt(HD))
        wk = w_in[E + rows, :]
        wv = w_in[2 * E + rows, :]
        w_loc = np.concatenate([wq, wk, wv], axis=0)          # [768, E]
        wqkv = np.ascontiguousarray(
            w_loc.T.reshape(8, 128, 768)).astype(ml_dtypes.bfloat16)

        xT = np.ascontiguousarray(
            q32[b].T.reshape(8, 128, T)).astype(ml_dtypes.bfloat16)

        wo_loc = np.ascontiguousarray(
            w_out[:, rows].T.reshape(2, 128, E)).astype(ml_dtypes.bfloat16)

        biasd = np.empty((HL, 128, 640), ml_dtypes.bfloat16)
        for hl in range(HL):
            s = slopes[4 * hq + hl]
            eb = np.where(allowed, np.exp(-s * (cc - jj).astype(np.float64)), 0.0)
            biasd[hl] = eb.astype(ml_dtypes.bfloat16)

        in_maps.append({"xT": xT, "wqkv": wqkv, "wo": wo_loc, "biasd": biasd})
    return in_maps


def _assemble(results):
    out = np.zeros((B, T, E), np.float32)
    for c in range(NCORES):
        b = c // 4
        part = np.asarray(results[c]["outT"]).astype(np.float32)  # [8,128,T]
        out[b] += part.reshape(E, T).T
    return out


def kernel(query, in_proj_weight, out_proj_weight, num_heads, sliding_window_size):
    assert int(num_heads) == H and int(sliding_window_size) == W
    assert query.shape == (B, T, E)
    if "nc" not in _CACHE:
        _CACHE["nc"] = _build()
    in_maps = _host_inputs(query, in_proj_weight, out_proj_weight)
    res = run_bass_kernel_spmd(_CACHE["nc"], in_maps, list(range(NCORES))).results
    return _assemble(res)
